# revision 14
# baseline (speedup 1.0000x reference)
"""EntityEncoder (gnn_message_passing) Trainium2 kernel — 8-core SPMD, v2.

Strategy: edges pre-partitioned on host into 8 contiguous entity-aligned
shards (entity_indices sorted => no cross-core collectives). Per core,
segments LPT-packed into 10 blocks of <=128 segments / <=1280 edges.

v2 changes vs v1:
  - all embedding streams converted to bf16 on host (halves HBM reads)
  - count embeddings gathered on host into a 4th edge stream (removes
    one-hot count/prompt vector work and the count-table matmul chain)
  - prompt/count scorer contributions folded on host into one per-edge
    scalar (exp bias)
  - two-phase device schedule: aggregation (one-hot matmuls + PE
    transposes into resident transposed aggregates), then projection
    with weight-stationary matmuls (one LDW per 1280 streamed cols)
  - outputs written transposed [OUT, E_PAD] in bf16; bias + transpose
    + scatter on host
"""
import sys
import numpy as np
import ml_dtypes

for _p in ("/root/.axon_site", "/root/.axon_site/_ro/trn_rl_repo",
           "/root/.axon_site/_ro/pypackages"):
    if _p not in sys.path:
        sys.path.append(_p)

import bass_rust
import concourse.bass as bass
import concourse.mybir as mybir
import concourse.tile as tile
from concourse.vector_clock import ScopedClock
from contextlib import ExitStack

BF16 = ml_dtypes.bfloat16
dt = mybir.dt
Alu = mybir.AluOpType
Act = mybir.ActivationFunctionType

# problem shape (hardcoded per contest contract)
N_CORES = 8
N = 100_000
P = 64
E = 10_000
D = 768
C = 1000
OUT = 5120
# per-core packing
NBLK = 10
SPB = 128                # segs per block
CH = 10                  # chunks (of 128 edges) per block
EPB = CH * 128           # edges per block = 1280
NL = NBLK * EPB          # 12800 edge slots per core
E_PAD = NBLK * SPB       # 1280 seg slots per core
KC_R = 12                # rel|cnt contraction chunks (1536/128)
KC_E = 6                 # ent contraction chunks (768/128)
DC = 832                 # cnt stream width: 768 emb + ones col at 768
NOT = OUT // 128         # 40 output tiles of 128 cols
PAD_SEG = 999.0


class _TileContextSplitDrain(tile.TileContext):
    """This container's walrus accepts only ONE sync wait per instruction
    ("Too many sync wait commands" in setupSyncWait). Split every extra wait
    onto a standalone same-engine NoOp placed immediately before the
    instruction — identical semantics, one wait per instruction."""

    def _lower_ordered_insts(self, ordered):
        for insts in ordered.values():
            if not any(
                i.sync_info is not None and len(i.sync_info.on_wait) > 1
                for i in insts
            ):
                continue
            new = []
            for inst in insts:
                si = inst.sync_info
                if si is not None and len(si.on_wait) > 1:
                    waits = list(si.on_wait)
                    for w in waits[:-1]:
                        nop = bass_rust.InstNoOp(
                            name=self.nc.get_next_instruction_name(),
                            ins=[], outs=[])
                        nop.engine = inst.engine
                        nop.sync_info = bass_rust.SyncInfo(
                            on_wait=[w], on_update=[])
                        new.append(nop)
                    si.on_wait = waits[-1:]
                new.append(inst)
            insts[:] = new
        return super()._lower_ordered_insts(ordered)

    def _drain_and_barrier(self, tick_clock, wait_clock):
        nc = self.nc
        drain_inst = nc.sync.drain()
        wait_clock.add_sem_waits(
            drain_inst.ins, ScopedClock({None: tick_clock.global_clock})
        )
        si = drain_inst.ins.sync_info
        if si is not None and len(si.on_wait) > 1:
            waits = list(si.on_wait)
            si.on_wait = waits[:1]
            for w in waits[1:]:
                n = nc.sync.nop()
                n.ins.sync_info = bass_rust.SyncInfo(on_wait=[w], on_update=[])
        nc.all_engine_barrier()
        assert self.sems is not None
        popped = nc._tile_sem_poison_stack.pop()
        assert popped is self._sem_poison
        nc.clear_and_free_semaphores(list(self.sems.allocated().values()))
        nc.all_engine_barrier()


# --------------------------------------------------------------------------
# host-side sharding / packing
# --------------------------------------------------------------------------

def _shard_and_pack(entity_indices):
    Nn = entity_indices.shape[0]
    starts = np.searchsorted(entity_indices, np.arange(E + 1))
    ideal = (np.arange(1, N_CORES) * Nn) // N_CORES
    ent_bnd = [0]
    for t in ideal:
        s = int(np.searchsorted(starts, t))
        if s > 0 and abs(int(starts[s - 1]) - int(t)) < abs(int(starts[s]) - int(t)):
            s -= 1
        ent_bnd.append(s)
    ent_bnd.append(E)

    cores = []
    for c in range(N_CORES):
        e_lo, e_hi = ent_bnd[c], ent_bnd[c + 1]
        segs = np.arange(e_lo, e_hi)
        sizes = (starts[e_lo + 1 : e_hi + 1] - starts[e_lo:e_hi]).astype(np.int64)
        n_edges = int(sizes.sum())
        assert e_hi - e_lo <= E_PAD and n_edges <= NL
        order = np.argsort(-sizes, kind="stable")
        blk_edges = [0] * NBLK
        blk_nseg = [0] * NBLK
        blk_segs = [[] for _ in range(NBLK)]
        for idx in order:
            sz = int(sizes[idx])
            best = -1
            for b in sorted(range(NBLK), key=lambda b: blk_edges[b]):
                if blk_nseg[b] < SPB and blk_edges[b] + sz <= EPB:
                    best = b
                    break
            assert best >= 0, "block packing overflow"
            blk_segs[best].append(int(segs[idx]))
            blk_edges[best] += sz
            blk_nseg[best] += 1
        perm = np.full(NL, -1, dtype=np.int64)
        seg_local = np.full(NL, PAD_SEG, dtype=np.float32)
        row2seg = np.full(E_PAD, -1, dtype=np.int64)
        inv_cnt = np.zeros(E_PAD, dtype=np.float32)
        for b in range(NBLK):
            pos = b * EPB
            for j, s in enumerate(blk_segs[b]):
                row = b * SPB + j
                row2seg[row] = s
                n = int(starts[s + 1] - starts[s])
                if n > 0:
                    inv_cnt[row] = 1.0 / n
                perm[pos : pos + n] = np.arange(starts[s], starts[s + 1])
                seg_local[pos : pos + n] = float(j)
                pos += n
        cores.append(dict(perm=perm, seg_local=seg_local, row2seg=row2seg,
                          inv_cnt=inv_cnt))
    return cores


# --------------------------------------------------------------------------
# device kernel
# --------------------------------------------------------------------------

def _build_nc():
    nc = bass.Bass("TRN2", target_bir_lowering=False, debug=False,
                   num_devices=N_CORES)

    f32, bf, i32 = dt.float32, dt.bfloat16, dt.int32
    din = lambda n, s, d=f32: nc.dram_tensor(n, s, d, kind="ExternalInput")
    mega_d = din("mega", [2 * NBLK, 128, (CH // 2) * 3 * D], bf)
    cnt_d = din("cnt", [2 * NBLK, 128, (CH // 2) * DC], bf)
    segl_d = din("segl", [NL])
    sc0_d = din("sc0", [NL])
    icnt_d = din("inv_cnt", [E_PAD])
    wsenr_d = din("wsenr", [128, 3 * D], bf)
    # tiled projector weights: [ot, k, 128, 128] (lhsT tiles)
    wtr_d = din("wtr", [NOT, KC_R, 128, 128], bf)
    wte_d = din("wte", [NOT, KC_E, 128, 128], bf)
    orelT_d = nc.dram_tensor("orelT", [OUT, E_PAD], bf, kind="ExternalOutput")
    oentT_d = nc.dram_tensor("oentT", [OUT, E_PAD], bf, kind="ExternalOutput")

    with _TileContextSplitDrain(nc) as tc, ExitStack() as es:
        const = es.enter_context(tc.tile_pool(name="const", bufs=1))
        accp = es.enter_context(tc.tile_pool(name="accp", bufs=1))

        # ---- constants ----
        iota_seg = const.tile([128, 128], bf)
        ident = const.tile([128, 128], bf)
        with tc.tile_pool(name="setup", bufs=1) as setup:
            iota_i = setup.tile([128, 128], i32)
            nc.gpsimd.iota(iota_i[:], pattern=[[1, 128]], base=0,
                           channel_multiplier=0)
            nc.vector.tensor_copy(iota_seg[:], iota_i[:])
            iota_ci = setup.tile([128, 1], i32)
            nc.gpsimd.iota(iota_ci[:], pattern=[[0, 1]], base=0,
                           channel_multiplier=1)
            iota_col = setup.tile([128, 1], f32)
            nc.vector.tensor_copy(iota_col[:], iota_ci[:])
            nc.vector.tensor_scalar(out=ident[:], in0=iota_seg[:],
                                    scalar1=iota_col[:],
                                    scalar2=None, op0=Alu.is_equal)
        wsenr = const.tile([128, 3 * D], bf)
        nc.sync.dma_start(wsenr[:], wsenr_d.ap())
        icnt_sb = const.tile([128, NBLK], f32)
        nc.sync.dma_start(
            icnt_sb[:], icnt_d.ap().rearrange("(b p) -> p b", p=128)
        )
        invd_sb = accp.tile([128, NBLK], f32)

        # resident transposed aggregates: ATr[k] = [128 (k-dim), 1280 (seg)]
        ATr = [accp.tile([128, E_PAD], bf, name=f"ATr{k}", tag=f"ATr{k}")
               for k in range(KC_R)]
        ATe = [accp.tile([128, E_PAD], bf, name=f"ATe{k}", tag=f"ATe{k}")
               for k in range(KC_E)]

        HD = (CH // 2) * 3 * D  # half-block mega width (5 chunks x 2304)
        HDC = (CH // 2) * DC

        # ================= Phase A + staircase =================
        SC_OTS = list(range(NOT))  # all bg0 cells run during phase A
        GAP_AT = [0, 7, 14, 21, 28, 34, 40]  # SC_OTS slices per gap
        parts = None  # filled below

        with tc.tile_pool(name="edges", bufs=3) as edges, \
             tc.tile_pool(name="chunkp", bufs=4) as chunkp, \
             tc.tile_pool(name="ohp", bufs=12) as ohp, \
             tc.tile_pool(name="scrp", bufs=2) as scrp, \
             tc.tile_pool(name="evac", bufs=2) as evac, \
             tc.tile_pool(name="wpool", bufs=2) as wpool, \
             tc.tile_pool(name="outp", bufs=4) as outp, \
             tc.tile_pool(name="psagg", bufs=1, space="PSUM") as psagg, \
             tc.tile_pool(name="pp", bufs=2, space="PSUM") as pp, \
             tc.tile_pool(name="cellps", bufs=2, space="PSUM") as cellps:

            parts = (
                ("r", wtr_d, ATr, KC_R, orelT_d),
                ("e", wte_d, ATe, KC_E, oentT_d),
            )
            BGS = ((0, 512), (512, 1024), (1024, 1280))

            def emit_cell(pi, ot, bg, wt):
                tag, wt_d, ATl, KC, o_d = parts[pi]
                lo, hi = BGS[bg]
                w = hi - lo
                ps = cellps.tile([128, 512], f32, tag="cell")
                for k in range(KC):
                    nc.tensor.matmul(ps[:, 0:w], wt[:, k * 128 : (k + 1) * 128],
                                     ATl[k][:, lo:hi],
                                     start=(k == 0), stop=(k == KC - 1))
                stage = outp.tile([128, 512], bf, tag="cst")
                if (ot + bg) % 2 == 0:
                    nc.vector.tensor_copy(stage[:, 0:w], ps[:, 0:w])
                else:
                    nc.scalar.activation(stage[:, 0:w], ps[:, 0:w], Act.Copy)
                nc.scalar.dma_start(
                    o_d.ap()[ot * 128 : (ot + 1) * 128, lo:hi], stage[:, 0:w]
                )

            def load_wt(pi, ot):
                tag, wt_d, ATl, KC, o_d = parts[pi]
                wt = wpool.tile([128, KC * 128], bf, tag=f"wt{tag}")
                nc.gpsimd.dma_start(
                    wt[:], wt_d.ap()[ot].rearrange("k p c -> p k c")
                )
                return wt

            for b in range(NBLK):
                halves = []
                for hb in range(2):
                    r0 = b * EPB + hb * (EPB // 2)
                    r1 = r0 + EPB // 2
                    megah = edges.tile([128, HD], bf, tag="megah")
                    nc.sync.dma_start(megah[:], mega_d.ap()[2 * b + hb])
                    cnth = edges.tile([128, HDC], bf, tag="cnth")
                    nc.scalar.dma_start(cnth[:], cnt_d.ap()[2 * b + hb])
                    slh = edges.tile([128, CH // 2], f32, tag="slh")
                    nc.sync.dma_start(
                        slh[:], segl_d.ap()[r0:r1].rearrange("(p j) -> p j", j=CH // 2))
                    sch = edges.tile([128, CH // 2], f32, tag="sch")
                    nc.sync.dma_start(
                        sch[:], sc0_d.ap()[r0:r1].rearrange("(p j) -> p j", j=CH // 2))
                    halves.append((megah, cnth, slh, sch))

                # ---- pass 1: rel|cnt aggregation (+denominator) ----
                ps_rc = psagg.tile([128, 2048], f32, tag="ps")
                ohs = []
                for j in range(CH):
                    megah, cnth, slh, sch = halves[j // 5]
                    jj = j % 5
                    mj = megah[:, jj * 3 * D : (jj + 1) * 3 * D]
                    rj = megah[:, jj * 3 * D + 2 * D : (jj + 1) * 3 * D]
                    cj = cnth[:, jj * DC : jj * DC + DC]
                    scr = scrp.tile([128, 3 * D], bf, tag="scr")
                    sa = chunkp.tile([128, 1], f32, tag="sa")
                    nc.vector.scalar_tensor_tensor(
                        out=scr[:], in0=mj, scalar=1.0, in1=wsenr[:],
                        op0=Alu.mult, op1=Alu.mult, accum_out=sa[:])
                    ex_ = chunkp.tile([128, 1], f32, tag="ex_")
                    nc.scalar.activation(ex_[:], sa[:], Act.Exp,
                                         bias=sch[:, jj : jj + 1])
                    oh = ohp.tile([128, 128], bf, tag="oh")
                    nc.vector.tensor_scalar(out=oh[:], in0=iota_seg[:],
                                            scalar1=slh[:, jj : jj + 1],
                                            scalar2=None, op0=Alu.is_equal)
                    ohs.append(oh)
                    ohx = chunkp.tile([128, 128], bf, tag="ohx")
                    nc.vector.tensor_scalar(out=ohx[:], in0=iota_seg[:],
                                            scalar1=slh[:, jj : jj + 1],
                                            scalar2=ex_[:],
                                            op0=Alu.is_equal, op1=Alu.mult)
                    st, sp = (j == 0), (j == CH - 1)
                    nc.tensor.matmul(ps_rc[:, 0:512], ohx[:], rj[:, 0:512],
                                     start=st, stop=sp)
                    nc.tensor.matmul(ps_rc[:, 512:768], ohx[:], rj[:, 512:768],
                                     start=st, stop=sp)
                    nc.tensor.matmul(ps_rc[:, 1024:1536], ohx[:], cj[:, 0:512],
                                     start=st, stop=sp)
                    nc.tensor.matmul(ps_rc[:, 1536:1856], ohx[:], cj[:, 512:832],
                                     start=st, stop=sp)

                # epilogue 1: invd + normalized rel|cnt evac + transposes
                dmx = chunkp.tile([128, 1], f32, tag="dmx")
                nc.vector.tensor_scalar(out=dmx[:], in0=ps_rc[:, 1792:1793],
                                        scalar1=1e-30, scalar2=None, op0=Alu.max)
                nc.vector.reciprocal(invd_sb[:, b : b + 1], dmx[:])
                rcsb = evac.tile([128, 2 * D], bf, tag="rcsb")
                nc.scalar.activation(rcsb[:, 0:768], ps_rc[:, 0:768], Act.Copy,
                                     scale=invd_sb[:, b : b + 1])
                nc.scalar.activation(rcsb[:, 768:1536], ps_rc[:, 1024:1792],
                                     Act.Copy, scale=invd_sb[:, b : b + 1])

                bs = slice(b * 128, (b + 1) * 128)
                for k in range(KC_R):
                    pt = pp.tile([128, 512], bf, tag="pp")
                    nc.tensor.transpose(pt[:, 0:128],
                                        rcsb[:, k * 128 : (k + 1) * 128],
                                        ident[:])
                    nc.scalar.activation(ATr[k][:, bs], pt[:, 0:128],
                                         Act.Copy)

                # ---- pass 2: ent aggregation (psum banks reused) ----
                ps_ed = psagg.tile([128, 2048], f32, tag="ps")
                for j in range(CH):
                    megah, cnth, slh, sch = halves[j // 5]
                    jj = j % 5
                    ej = megah[:, jj * 3 * D : jj * 3 * D + D]
                    st, sp = (j == 0), (j == CH - 1)
                    nc.tensor.matmul(ps_ed[:, 0:512], ohs[j][:], ej[:, 0:512],
                                     start=st, stop=sp)
                    nc.tensor.matmul(ps_ed[:, 512:768], ohs[j][:], ej[:, 512:768],
                                     start=st, stop=sp)
                ohs = None
                edsb = evac.tile([128, D], bf, tag="edsb")
                nc.scalar.activation(edsb[:], ps_ed[:, 0:768], Act.Copy,
                                     scale=icnt_sb[:, b : b + 1])
                for k in range(KC_E):
                    pt = pp.tile([128, 512], bf, tag="pp")
                    nc.tensor.transpose(pt[:, 0:128],
                                        edsb[:, k * 128 : (k + 1) * 128],
                                        ident[:])
                    nc.scalar.activation(ATe[k][:, bs], pt[:, 0:128],
                                         Act.Copy)

                # ---- staircase: bg0 cells for 3 ots per gap after block 3 ----
                if 4 <= b <= 9:
                    for ot in SC_OTS[GAP_AT[b - 4] : GAP_AT[b - 3]]:
                        wtr_t = load_wt(0, ot)
                        wte_t = load_wt(1, ot)
                        emit_cell(0, ot, 0, wtr_t)
                        emit_cell(1, ot, 0, wte_t)

            # ---- tail: remaining cells ----
            for ot in range(NOT):
                wtr_t = load_wt(0, ot)
                wte_t = load_wt(1, ot)
                bgs = (1, 2) if ot in SC_OTS else (0, 1, 2)
                for bg in bgs:
                    emit_cell(0, ot, bg, wtr_t)
                for bg in bgs:
                    emit_cell(1, ot, bg, wte_t)
    return nc


_NC_CACHE = None


def _get_nc():
    global _NC_CACHE
    if _NC_CACHE is None:
        _NC_CACHE = _build_nc()
    return _NC_CACHE


# --------------------------------------------------------------------------
# entry point
# --------------------------------------------------------------------------

def kernel(prompt_embs, entity_embs, neighbor_embs, relation_embs,
           count_table, scorer_W, scorer_b, rel_W, rel_b, ent_W, ent_b,
           counts, prompt_indices, entity_indices):
    from concourse.bass_utils import run_bass_kernel_spmd

    prompt_embs = np.asarray(prompt_embs, dtype=np.float32)
    entity_embs = np.asarray(entity_embs, dtype=np.float32)
    neighbor_embs = np.asarray(neighbor_embs, dtype=np.float32)
    relation_embs = np.asarray(relation_embs, dtype=np.float32)
    count_table = np.asarray(count_table, dtype=np.float32)
    scorer_W = np.asarray(scorer_W, dtype=np.float32)
    scorer_b = np.asarray(scorer_b, dtype=np.float32)
    rel_W = np.asarray(rel_W, dtype=np.float32)
    rel_b = np.asarray(rel_b, dtype=np.float32)
    ent_W = np.asarray(ent_W, dtype=np.float32)
    ent_b = np.asarray(ent_b, dtype=np.float32)
    counts = np.asarray(counts)
    prompt_indices = np.asarray(prompt_indices)
    entity_indices = np.asarray(entity_indices)

    cores = _shard_and_pack(entity_indices)

    # replicated (weight-derived) host prep
    w = scorer_W[0]
    w1, w2, w3, w4, w5 = (w[i * D : (i + 1) * D] for i in range(5))
    pscore = (prompt_embs * w1[None, :]).sum(1) + scorer_b[0]     # fold bias
    cscore = (count_table * w5[None, :]).sum(1)
    wsenr = np.broadcast_to(
        np.concatenate([w2, w3, w4]).astype(BF16), (128, 3 * D)).copy()
    # tiled lhsT weight tiles: wtr[ot, k, kl, ol] = rel_W[ot*128+ol, k*128+kl]
    wtr = np.ascontiguousarray(
        rel_W.T.reshape(KC_R, 128, NOT, 128).transpose(2, 0, 1, 3)
    ).astype(BF16)
    wte = np.ascontiguousarray(
        ent_W.T.reshape(KC_E, 128, NOT, 128).transpose(2, 0, 1, 3)
    ).astype(BF16)

    mega16 = np.empty((N, 3 * D), dtype=BF16)
    mega16[:, 0:D] = entity_embs.astype(BF16)
    mega16[:, D : 2 * D] = neighbor_embs.astype(BF16)
    mega16[:, 2 * D :] = relation_embs.astype(BF16)
    cnt16 = np.zeros((N, DC), dtype=BF16)
    cnt16[:, 0:D] = count_table.astype(BF16)[counts]  # [N, D] gather
    cnt16[:, D] = BF16(1.0)                           # denominator ones col
    sc0_full = (pscore[prompt_indices] + cscore[counts]).astype(np.float32)

    in_maps = []
    for core in cores:
        perm = core["perm"]
        valid = perm >= 0
        src = np.where(valid, perm, 0)

        def take2d(a16):
            out = a16[src]
            out[~valid] = 0.0
            return np.ascontiguousarray(out)

        sc0 = sc0_full[src]
        sc0[~valid] = 0.0

        in_maps.append(dict(
            mega=take2d(mega16).reshape(2 * NBLK, 128, (CH // 2) * 3 * D),
            cnt=take2d(cnt16).reshape(2 * NBLK, 128, (CH // 2) * DC),
            segl=core["seg_local"], sc0=np.ascontiguousarray(sc0),
            inv_cnt=core["inv_cnt"],
            wsenr=wsenr, wtr=wtr, wte=wte,
        ))

    nc = _get_nc()
    res = run_bass_kernel_spmd(nc, in_maps, list(range(N_CORES)))

    rel_out = np.zeros((E, OUT), np.float32)
    ent_out = np.zeros((E, OUT), np.float32)
    for c, core in enumerate(cores):
        rows = core["row2seg"]
        mask = rows >= 0
        rel_out[rows[mask]] = res.results[c]["orelT"].T[mask].astype(np.float32)
        ent_out[rows[mask]] = res.results[c]["oentT"].T[mask].astype(np.float32)
    rel_out += rel_b[None, :]
    ent_out += ent_b[None, :]
    return rel_out, ent_out
